# revision 11
# baseline (speedup 1.0000x reference)
"""Trainium2 Bass kernel for nn_DWAModel (moe_routing).

8 cores: core c -> example e=c//2, stripe s=c%2 (owns q-tiles {2i+s}).
Blocks are token-striped data-parallel with a pair-AllGather of h_A (f32,
for routing precision); the expert pool is replicated and gathered on-device
via top-8 routing + indirect DMA; the LM head is vocab-sharded (4000/core)
after an 8-way AllGather of h_out.

Activations are feature-major ([feat_part, token_free]); LN across the
feature (partition) axis uses ones-matmul broadcast stats; causal masking is
data-driven (mask-matmul PSUM accumulation) so one SPMD program fits all
cores; s-dependent selections are encoded in per-core 0/1 input scalars.
"""

import sys

sys.path.insert(0, "/opt/trn_rl_repo")

from contextlib import ExitStack

import numpy as np
import ml_dtypes

import concourse.bass as bass
import concourse.mybir as mybir
from concourse import bacc, tile
from concourse.bass_utils import run_bass_kernel_spmd

P = 128
T = 1024
D = 512
DT = 4
NH = 8
DH = 64
FF = 2048
FT = FF // P
NPOOL = 512
TOPK = 8
VOC = 32000
NCORES = 8
VS = VOC // NCORES
VCH = 500
NVC = VS // VCH
TOWN = 512
NEG = -300.0
EPS = 1e-5

bf16 = mybir.dt.bfloat16
f32 = mybir.dt.float32
u32 = mybir.dt.uint32
AX = mybir.AxisListType
OP = mybir.AluOpType
AF = mybir.ActivationFunctionType

_cache = {}


def build(lambda_f: float, warmup: bool):
    nc = bacc.Bacc(None, target_bir_lowering=False, debug=False)

    def din(name, shape, dt=bf16):
        return nc.declare_dram_parameter(name, list(shape), dt, isOutput=False)

    x_fm = din("x_fm", [P, DT, T])
    x_own = din("x_own", [P, DT, TOWN], f32)
    wqkv_a = din("wqkv_a", [P, DT, 3 * D])
    wo_a = din("wo_a", [P, DT, D])
    wf1_a = din("wf1_a", [P, DT, FF])
    wf2_a = din("wf2_a", [P, FT, D])
    wqkv_b = din("wqkv_b", [P, DT, 3 * D])
    wo_b = din("wo_b", [P, DT, D])
    wf1_b = din("wf1_b", [P, DT, FF])
    wf2_b = din("wf2_b", [P, FT, D])
    wq_ret = din("wq_ret", [P, DT, 128], f32)
    keys_t = din("keys_t", [P, NPOOL], f32)
    w_base_r = din("w_base_r", [P, DT, D], f32)
    pool_r = din("pool_r", [NPOOL * D, D])
    lm_w = din("lm_w", [P, DT, VS])
    mask_own_i = din("mask_own", [P, TOWN])
    mask_oth_i = din("mask_oth", [P, TOWN])
    selv = din("selv", [P, 2], f32)
    iota_pj = din("iota_pj", [P, DT], f32)
    id128 = din("id128", [P, P])

    logits_out = nc.declare_dram_parameter("logits_out", [NCORES * TOWN, VS], f32, isOutput=True)
    DBG = bool(__import__("os").environ.get("KBDBG"))
    if DBG:
        dbg_hsel = nc.declare_dram_parameter("dbg_hsel", [P, DT, TOWN], bf16, isOutput=True)
        dbg_wacc = nc.declare_dram_parameter("dbg_wacc", [P, DT, D], f32, isOutput=True)
        dbg_addr = nc.declare_dram_parameter("dbg_addr", [P, TOPK, DT], u32, isOutput=True)
        dbg_hmid = nc.declare_dram_parameter("dbg_hmid", [P, DT, T], bf16, isOutput=True)
        dbg_hb = nc.declare_dram_parameter("dbg_hb", [P, DT, TOWN], bf16, isOutput=True)
        dbg_g = nc.declare_dram_parameter("dbg_g", [P, DT, D], bf16, isOutput=True)
    alphas_out = nc.declare_dram_parameter("alphas_out", [1, TOPK], f32, isOutput=True)
    idx_out = nc.declare_dram_parameter("idx_out", [1, TOPK], u32, isOutput=True)

    agA_in = nc.dram_tensor("agA_in", [P, DT, TOWN], f32)
    agA_out = nc.dram_tensor("agA_out", [2, P, DT, TOWN], f32)
    agB_in = nc.dram_tensor("agB_in", [P, DT, TOWN], bf16)
    agB_out = nc.dram_tensor("agB_out", [NCORES, P, DT, TOWN], bf16)

    ctx = ExitStack()
    with tile.TileContext(nc) as tc, ctx:
        cp = ctx.enter_context(tc.tile_pool(name="const", bufs=1))
        wp = ctx.enter_context(tc.tile_pool(name="work", bufs=1))
        ep = ctx.enter_context(tc.tile_pool(name="esb", bufs=2))
        gp = ctx.enter_context(tc.tile_pool(name="gat", bufs=2))
        op_ = ctx.enter_context(tc.tile_pool(name="oscp", bufs=1))
        lp = ctx.enter_context(tc.tile_pool(name="lst", bufs=3))
        psA = ctx.enter_context(tc.tile_pool(name="psA", bufs=3, space="PSUM"))
        psS = ctx.enter_context(tc.tile_pool(name="psS", bufs=2, space="PSUM"))
        psO = ctx.enter_context(tc.tile_pool(name="psO", bufs=2, space="PSUM"))

        _ev = [0]

        def evac(out, in_, scale=None):
            if scale is not None:
                nc.scalar.mul(out, in_, scale)
                return
            if _ev[0] % 2 == 0:
                nc.scalar.copy(out, in_)
            else:
                nc.vector.tensor_copy(out=out, in_=in_)
            _ev[0] += 1

        def ldconst(name, handle, shape, dt=bf16, pool=None):
            t = (pool or cp).tile(list(shape), dt, tag=name)
            nc.sync.dma_start(out=t[:], in_=handle[:])
            return t

        xfm = ldconst("xfm", x_fm, [P, DT, T], pool=wp)
        xown = ldconst("xown", x_own, [P, DT, TOWN], f32, pool=wp)
        m_own = ldconst("m_own", mask_own_i, [P, TOWN])
        m_oth = ldconst("m_oth", mask_oth_i, [P, TOWN])
        sel = ldconst("sel", selv, [P, 2], f32)
        iopj = ldconst("iopj", iota_pj, [P, DT], f32)
        idm = ldconst("idm", id128, [P, P])
        wq_sb = ldconst("wq_sb", wq_ret, [P, DT, 128], f32)
        keys_sb = ldconst("keys_sb", keys_t, [P, NPOOL], f32)

        ones_pp = cp.tile([P, P], bf16, tag="ones_pp")
        nc.vector.memset(ones_pp[:], 1.0)
        ones_col_f = cp.tile([P, 1], f32, tag="ones_col_f")
        nc.vector.memset(ones_col_f[:], 1.0)
        ones_row_f = cp.tile([1, P], f32, tag="ones_row_f")
        nc.vector.memset(ones_row_f[:], 1.0)
        epst = cp.tile([P, 1], f32, tag="epst")
        nc.vector.memset(epst[:], EPS)

        def layernorm(src, tcols, xn):
            """src [P, DT, tcols] bf16 -> xn (LN across the feature axis)."""
            for cc in range(tcols // 512):
                cs = slice(cc * 512, cc * 512 + 512)
                sq = wp.tile([P, DT, 512], bf16, tag="ln_sq")
                nc.vector.tensor_tensor(out=sq[:], in0=src[:, :, cs], in1=src[:, :, cs], op=OP.mult)
                ps_m = psA.tile([P, 512], f32, tag="ps")
                ps_s = psA.tile([P, 512], f32, tag="ps")
                for dt in range(DT):
                    nc.tensor.matmul(out=ps_m[:], lhsT=ones_pp[:], rhs=src[:, dt, cs],
                                     start=(dt == 0), stop=(dt == DT - 1))
                for dt in range(DT):
                    nc.tensor.matmul(out=ps_s[:], lhsT=ones_pp[:], rhs=sq[:, dt, :],
                                     start=(dt == 0), stop=(dt == DT - 1))
                mu = wp.tile([P, 512], f32, tag="ln_mu")
                nc.scalar.mul(mu[:], ps_m[:], 1.0 / D)
                mu2 = wp.tile([P, 512], f32, tag="ln_mu2")
                nc.vector.tensor_tensor(out=mu2[:], in0=mu[:], in1=mu[:], op=OP.mult)
                nc.vector.scalar_tensor_tensor(out=mu2[:], in0=ps_s[:], scalar=1.0 / D,
                                               in1=mu2[:], op0=OP.mult, op1=OP.subtract)
                sd = wp.tile([P, 512], f32, tag="ln_sd")
                nc.scalar.activation(sd[:], mu2[:], AF.Sqrt, bias=epst[:])
                nc.vector.reciprocal(sd[:], sd[:])
                for dt in range(DT):
                    t0 = wp.tile([P, 512], f32, tag="ln_t0")
                    nc.vector.tensor_tensor(out=t0[:], in0=src[:, dt, cs], in1=mu[:], op=OP.subtract)
                    nc.vector.tensor_tensor(out=xn[:, dt, cs], in0=t0[:], in1=sd[:], op=OP.mult)

        def transformer_block(src_bf, res, wqkv_sb, wo_sb, wf1_sb, wf2_sb, h2_out, h2bf_out):
            xn = wp.tile([P, DT, T], bf16, tag="xn")
            layernorm(src_bf, T, xn)

            qf = wp.tile([P, DT, TOWN], bf16, tag="qf")
            kf = wp.tile([P, DT, T], bf16, tag="kf")
            vaug = wp.tile([P, 8, NH, DH + 1], bf16, tag="vaug")
            nc.vector.memset(vaug[:, :, :, DH:DH + 1], 1.0)
            for fc in range(DT):
                ps = psA.tile([P, 512], f32, tag="ps")
                for dt in range(DT):
                    nc.tensor.matmul(out=ps[:], lhsT=wqkv_sb[:, dt, fc * P:(fc + 1) * P],
                                     rhs=xn[:, dt, 0:TOWN], start=(dt == 0), stop=(dt == DT - 1))
                evac(qf[:, fc, :], ps[:], scale=1.0 / np.sqrt(DH))
            for fc in range(DT):
                for cc in range(2):
                    ps = psA.tile([P, 512], f32, tag="ps")
                    for dt in range(DT):
                        nc.tensor.matmul(out=ps[:], lhsT=wqkv_sb[:, dt, D + fc * P:D + (fc + 1) * P],
                                         rhs=xn[:, dt, cc * 512:(cc + 1) * 512],
                                         start=(dt == 0), stop=(dt == DT - 1))
                    evac(kf[:, fc, cc * 512:(cc + 1) * 512], ps[:])
            for tch in range(8):
                ps = psA.tile([P, 512], f32, tag="ps")
                for dt in range(DT):
                    nc.tensor.matmul(out=ps[:], lhsT=xn[:, dt, tch * P:(tch + 1) * P],
                                     rhs=wqkv_sb[:, dt, 2 * D:3 * D],
                                     start=(dt == 0), stop=(dt == DT - 1))
                nc.vector.tensor_copy(out=vaug[:, tch, :, 0:DH],
                                      in_=ps[:].rearrange("p (h d) -> p h d", h=NH))

            o_cat = wp.tile([P, DT, TOWN], bf16, tag="ocat")
            for h in range(NH):
                r0 = (h % 2) * DH
                fc = h // 2
                e_sb = ep.tile([P, 8, 512], bf16, tag="esb")
                for j in range(4):  # own-half k-tiles
                    nj = 512 - 128 * j
                    ps = psS.tile([P, 512], f32, tag="ps")
                    nc.tensor.matmul(out=ps[:, j * 128:512], lhsT=idm[:], rhs=m_own[:, 0:nj],
                                     start=True, stop=False)
                    nc.tensor.matmul(out=ps[:, j * 128:512],
                                     lhsT=kf[r0:r0 + DH, fc, j * 128:(j + 1) * 128],
                                     rhs=qf[r0:r0 + DH, fc, j * 128:512],
                                     start=False, stop=True)
                    nc.scalar.activation(e_sb[:, j, j * 128:512], ps[:, j * 128:512], AF.Exp)
                for j in range(4):  # other-half k-tiles (kt' = 4+j)
                    nj = 512 - 128 * j
                    ps = psS.tile([P, 512], f32, tag="ps")
                    nc.tensor.matmul(out=ps[:, j * 128:512], lhsT=idm[:], rhs=m_oth[:, 0:nj],
                                     start=True, stop=False)
                    nc.tensor.matmul(out=ps[:, j * 128:512],
                                     lhsT=kf[r0:r0 + DH, fc, TOWN + j * 128:TOWN + (j + 1) * 128],
                                     rhs=qf[r0:r0 + DH, fc, j * 128:512],
                                     start=False, stop=True)
                    nc.scalar.activation(e_sb[:, 4 + j, j * 128:512], ps[:, j * 128:512], AF.Exp)
                po = psO.tile([DH + 1, 512], f32, tag="po")
                order = [(0, 0), (4, 0), (1, 128), (5, 128), (2, 256), (6, 256), (3, 384), (7, 384)]
                for n, (kt, c0) in enumerate(order):
                    nc.tensor.matmul(out=po[:, c0:512], lhsT=vaug[:, kt, h, :],
                                     rhs=e_sb[:, kt, c0:512],
                                     start=(n == 0), stop=(n == len(order) - 1))
                # rows 0..63 of po = unnormalized o; row 64 = sum(e) (ones column)
                oa = op_.tile([DH + 1, 512], f32, tag="oa")
                nc.scalar.copy(oa[:], po[:])
                den0 = op_.tile([1, 512], f32, tag="den0")
                nc.sync.dma_start(out=den0[:], in_=oa[DH:DH + 1, :])
                nc.vector.reciprocal(den0[:], den0[:])
                rb = op_.tile([DH, 512], f32, tag="rb")
                nc.gpsimd.partition_broadcast(rb[:], den0[:], channels=DH)
                osc = op_.tile([DH, 512], bf16, tag="osc")
                nc.vector.tensor_tensor(out=osc[:], in0=oa[0:DH, :], in1=rb[:], op=OP.mult)
                nc.sync.dma_start(out=o_cat[r0:r0 + DH, fc, :], in_=osc[:])

            h1 = wp.tile([P, DT, TOWN], f32, tag="h1")
            for ec in range(DT):
                ps = psA.tile([P, 512], f32, tag="ps")
                for dt in range(DT):
                    nc.tensor.matmul(out=ps[:], lhsT=wo_sb[:, dt, ec * P:(ec + 1) * P],
                                     rhs=o_cat[:, dt, :], start=(dt == 0), stop=(dt == DT - 1))
                nc.vector.tensor_tensor(out=h1[:, ec, :], in0=ps[:], in1=res[:, ec, :], op=OP.add)

            h1bf = wp.tile([P, DT, TOWN], bf16, tag="h1bf")
            for ec in range(DT):
                evac(h1bf[:, ec, :], h1[:, ec, :])
            xn2 = wp.tile([P, DT, TOWN], bf16, tag="xn2")
            layernorm(h1bf, TOWN, xn2)
            ubf = wp.tile([P, FT, 512], bf16, tag="xn")  # reuses xn slot (dead by now)
            for fc in range(FT):
                ps = psA.tile([P, 512], f32, tag="ps")
                for dt in range(DT):
                    nc.tensor.matmul(out=ps[:], lhsT=wf1_sb[:, dt, fc * P:(fc + 1) * P],
                                     rhs=xn2[:, dt, :], start=(dt == 0), stop=(dt == DT - 1))
                nc.scalar.activation(ubf[:, fc, :], ps[:], AF.Gelu_apprx_tanh)
            for ec in range(DT):
                ps = psA.tile([P, 512], f32, tag="ps")
                for kt in range(FT):
                    nc.tensor.matmul(out=ps[:], lhsT=wf2_sb[:, kt, ec * P:(ec + 1) * P],
                                     rhs=ubf[:, kt, :], start=(kt == 0), stop=(kt == FT - 1))
                nc.vector.tensor_tensor(out=h2_out[:, ec, :], in0=ps[:], in1=h1[:, ec, :], op=OP.add)
                evac(h2bf_out[:, ec, :], h2_out[:, ec, :])

        # ---------------- Block A ----------------
        wqkvA = ldconst("wqkv", wqkv_a, [P, DT, 3 * D])
        woA = ldconst("wo", wo_a, [P, DT, D])
        wf1A = ldconst("wf1", wf1_a, [P, DT, FF])
        wf2A = ldconst("wf2", wf2_a, [P, FT, D])

        h2A = wp.tile([P, DT, TOWN], f32, tag="h2")
        hA_bf = wp.tile([P, DT, T], bf16, tag="hA_bf")
        transformer_block(xfm, xown, wqkvA, woA, wf1A, wf2A, h2A, hA_bf[:, :, 0:TOWN])

        # ---------------- pair-AllGather of h_A (f32) ----------------
        nc.sync.dma_start(out=agA_in[:], in_=h2A[:])
        nc.gpsimd.collective_compute(
            "AllGather", OP.bypass,
            ins=[agA_in.ap().opt()], outs=[agA_out.ap().opt()],
            replica_groups=[[0, 1], [2, 3], [4, 5], [6, 7]],
        )
        slot0 = wp.tile([P, DT, TOWN], f32, tag="h1")      # chains after h1(A)
        slot1 = wp.tile([P, DT, TOWN], f32, tag="xown")    # chains after xown
        nc.sync.dma_start(out=slot0[:], in_=agA_out[0])
        nc.sync.dma_start(out=slot1[:], in_=agA_out[1])

        # z before slot0 is scaled in place
        zo = wp.tile([P, DT], f32, tag="zo")
        z0 = wp.tile([P, DT], f32, tag="z0")
        z1 = wp.tile([P, DT], f32, tag="z1")
        for dt in range(DT):
            nc.vector.tensor_reduce(out=zo[:, dt:dt + 1], in_=h2A[:, dt, :], axis=AX.X, op=OP.add)
            nc.vector.tensor_reduce(out=z0[:, dt:dt + 1], in_=slot0[:, dt, :], axis=AX.X, op=OP.add)
            nc.vector.tensor_reduce(out=z1[:, dt:dt + 1], in_=slot1[:, dt, :], axis=AX.X, op=OP.add)
        z = wp.tile([P, DT], f32, tag="z")
        nc.vector.scalar_tensor_tensor(out=z[:], in0=z0[:], scalar=sel[:, 0:1], in1=zo[:],
                                       op0=OP.mult, op1=OP.add)
        nc.vector.scalar_tensor_tensor(out=z[:], in0=z1[:], scalar=sel[:, 1:2], in1=z[:],
                                       op0=OP.mult, op1=OP.add)

        # other-half select into hA_bf[:, :, 512:]
        nc.vector.tensor_scalar(slot0[:], slot0[:], sel[:, 0:1], None, OP.mult)
        nc.vector.scalar_tensor_tensor(out=hA_bf[:, :, TOWN:T], in0=slot1[:], scalar=sel[:, 1:2],
                                       in1=slot0[:], op0=OP.mult, op1=OP.add)

        # ---------------- routing ----------------
        rep = wp.tile([P, 16], f32, tag="rep")
        if lambda_f != 0.0:
            psq = psS.tile([P, 1], f32, tag="ps")
            for dt in range(DT):
                nc.tensor.matmul(out=psq[:], lhsT=wq_sb[:, dt, :], rhs=z[:, dt:dt + 1],
                                 start=(dt == 0), stop=(dt == DT - 1))
            qsb = wp.tile([P, 1], f32, tag="qsb")
            nc.vector.tensor_copy(out=qsb[:], in_=psq[:])
            qsq = wp.tile([P, 1], f32, tag="qsq")
            nc.vector.tensor_tensor(out=qsq[:], in0=qsb[:], in1=qsb[:], op=OP.mult)
            ps_ss = psS.tile([1, 1], f32, tag="ps")
            nc.tensor.matmul(out=ps_ss[:], lhsT=qsq[:], rhs=ones_col_f[:], start=True, stop=True)
            sdq = wp.tile([1, 1], f32, tag="sdq")
            nc.scalar.activation(sdq[:], ps_ss[:], AF.Sqrt)
            nc.vector.reciprocal(sdq[:], sdq[:])
            rq2 = wp.tile([1, 1], f32, tag="rq2")
            nc.scalar.mul(rq2[:], sdq[:], float(lambda_f))
            ps_sc = psS.tile([1, NPOOL], f32, tag="ps")
            nc.tensor.matmul(out=ps_sc[:], lhsT=qsb[:], rhs=keys_sb[:], start=True, stop=True)
            sc = wp.tile([1, NPOOL], f32, tag="sc")
            nc.scalar.activation(sc[:], ps_sc[:], AF.Copy, scale=rq2[:])
            vals = wp.tile([1, TOPK], f32, tag="vals")
            idxu = wp.tile([1, TOPK], u32, tag="idxu")
            nc.vector.max_with_indices(vals[:], idxu[:], sc[:])
            alph = wp.tile([1, TOPK], f32, tag="alph")
            if warmup:
                nc.vector.memset(alph[:], 1.0 / TOPK)
            else:
                evx = wp.tile([1, TOPK], f32, tag="evx")
                asum = wp.tile([1, 1], f32, tag="asum")
                nc.scalar.activation(evx[:], vals[:], AF.Exp, accum_out=asum[:])
                nc.vector.reciprocal(asum[:], asum[:])
                nc.vector.tensor_scalar(alph[:], evx[:], asum[:], None, OP.mult)
            nc.sync.dma_start(out=alphas_out[:], in_=alph[:])
            nc.sync.dma_start(out=idx_out[:], in_=idxu[:])
            idxf = wp.tile([1, TOPK], f32, tag="idxf")
            nc.vector.tensor_copy(out=idxf[:], in_=idxu[:])
            catv = wp.tile([1, 16], f32, tag="catv")
            nc.vector.tensor_copy(out=catv[:, 0:8], in_=idxf[:])
            nc.vector.tensor_copy(out=catv[:, 8:16], in_=alph[:])
            ps_rep = psS.tile([P, 16], f32, tag="ps")
            nc.tensor.matmul(out=ps_rep[:], lhsT=ones_row_f[:], rhs=catv[:], start=True, stop=True)
            nc.vector.tensor_copy(out=rep[:], in_=ps_rep[:])
            addru = wp.tile([P, TOPK, DT], u32, tag="addru")
            for j in range(DT):
                addrf = wp.tile([P, TOPK], f32, tag="addrf")
                nc.vector.scalar_tensor_tensor(
                    out=addrf[:], in0=rep[:, 0:8], scalar=float(D), op0=OP.mult,
                    op1=OP.add, in1=iopj[:, j:j + 1].to_broadcast([P, TOPK]))
                nc.vector.tensor_copy(out=addru[:, :, j], in_=addrf[:])
        else:
            idxu = wp.tile([1, TOPK], u32, tag="idxu")
            nc.gpsimd.iota(idxu[:], pattern=[[1, TOPK]], base=0, channel_multiplier=0)
            alph = wp.tile([1, TOPK], f32, tag="alph")
            nc.vector.memset(alph[:], 1.0 / TOPK)
            nc.sync.dma_start(out=alphas_out[:], in_=alph[:])
            nc.sync.dma_start(out=idx_out[:], in_=idxu[:])
            nc.vector.memset(rep[:, 8:16], 1.0 / TOPK)
            addru = wp.tile([P, TOPK, DT], u32, tag="addru")
            nc.gpsimd.iota(addru[:], pattern=[[D, TOPK], [128, DT]], base=0, channel_multiplier=1)

        # ---------------- gather + W assembly ----------------
        wacc = wp.tile([P, DT, D], f32, tag="xown")  # chains after slot1
        nc.sync.dma_start(out=wacc[:], in_=w_base_r[:])
        for k in range(TOPK):
            g = gp.tile([P, DT, D], bf16, tag="g")
            if DBG and k == 0:
                g0hold = g
            for j in range(DT):
                nc.gpsimd.indirect_dma_start(
                    out=g[:, j, :], out_offset=None, in_=pool_r[:, :],
                    in_offset=bass.IndirectOffsetOnAxis(ap=addru[:, k, j:j + 1], axis=0))
            nc.vector.scalar_tensor_tensor(out=wacc[:], in0=g[:], scalar=rep[:, 8 + k:9 + k],
                                           in1=wacc[:], op0=OP.mult, op1=OP.add)
            if DBG and k == 0:
                nc.sync.dma_start(out=dbg_g[:], in_=g0hold[:])
        wg = wp.tile([P, DT, D], bf16, tag="wg")
        nc.scalar.copy(wg[:], wacc[:])
        if DBG:
            nc.sync.dma_start(out=dbg_hsel[:], in_=hA_bf[:, :, TOWN:T])
            nc.sync.dma_start(out=dbg_wacc[:], in_=wacc[:])
            nc.sync.dma_start(out=dbg_addr[:], in_=addru[:])

        # ---------------- h_mid ----------------
        hmid_bf = wp.tile([P, DT, T], bf16, tag="xfm")  # chains after xfm
        for ec in range(DT):
            for cc in range(2):
                ps = psA.tile([P, 512], f32, tag="ps")
                for j in range(DT):
                    nc.tensor.matmul(out=ps[:], lhsT=wg[:, j, ec * P:(ec + 1) * P],
                                     rhs=hA_bf[:, j, cc * 512:(cc + 1) * 512],
                                     start=(j == 0), stop=(j == DT - 1))
                evac(hmid_bf[:, ec, cc * 512:(cc + 1) * 512], ps[:])

        if DBG:
            nc.sync.dma_start(out=dbg_hmid[:], in_=hmid_bf[:])
        # ---------------- Block B ----------------
        wqkvB = ldconst("wqkv", wqkv_b, [P, DT, 3 * D])
        woB = ldconst("wo", wo_b, [P, DT, D])
        wf1B = ldconst("wf1", wf1_b, [P, DT, FF])
        wf2B = ldconst("wf2", wf2_b, [P, FT, D])

        h2B = wp.tile([P, DT, TOWN], f32, tag="h2")
        hB_bf = wp.tile([P, DT, TOWN], bf16, tag="hB_bf")
        transformer_block(hmid_bf, hmid_bf[:, :, 0:TOWN], wqkvB, woB, wf1B, wf2B, h2B, hB_bf)

        if DBG:
            nc.sync.dma_start(out=dbg_hb[:], in_=hB_bf[:])
        # ---------------- 8-way AllGather of h_out ----------------
        nc.sync.dma_start(out=agB_in[:], in_=hB_bf[:])
        nc.gpsimd.collective_compute(
            "AllGather", OP.bypass,
            ins=[agB_in.ap().opt()], outs=[agB_out.ap().opt()],
            replica_groups=[list(range(NCORES))],
        )

        # ---------------- LM head (vocab-sharded) ----------------
        for half in range(2):
            lm_h = cp.tile([P, DT, VS // 2], bf16, tag="wf1")  # chains after wf1(B)
            nc.sync.dma_start(out=lm_h[:], in_=lm_w[:, :, half * (VS // 2):(half + 1) * (VS // 2)])
            for r in range(NCORES):
                xr = gp.tile([P, DT, TOWN], bf16, tag="g")
                nc.sync.dma_start(out=xr[:], in_=agB_out[r])
                for tl in range(4):
                    for vc in range(NVC // 2):
                        ps = psA.tile([P, VCH], f32, tag="ps")
                        for dt in range(DT):
                            nc.tensor.matmul(out=ps[:], lhsT=xr[:, dt, tl * P:(tl + 1) * P],
                                             rhs=lm_h[:, dt, vc * VCH:(vc + 1) * VCH],
                                             start=(dt == 0), stop=(dt == DT - 1))
                        st = lp.tile([P, VCH], f32, tag="lstage")
                        evac(st[:], ps[:])
                        row0 = (r * 4 + tl) * P
                        col0 = half * (VS // 2) + vc * VCH
                        nc.sync.dma_start(out=logits_out[row0:row0 + P, col0:col0 + VCH],
                                          in_=st[:])

    nc.finalize()
    return nc


# ============================ host side ============================

def _sin_pos(t, d):
    pos = np.arange(t, dtype=np.float32)[:, None]
    i = np.arange(d // 2, dtype=np.float32)[None, :]
    ang = (pos / (10000.0 ** (2 * i / d))).astype(np.float32)
    return np.concatenate([np.sin(ang), np.cos(ang)], -1)[:, :d].astype(np.float32)


def _fm(a, dt=ml_dtypes.bfloat16):
    """[K, N] -> [P, K//P, N] (K on partitions in 128-tiles)."""
    k, n = a.shape
    return np.ascontiguousarray(
        np.asarray(a, np.float32).reshape(k // P, P, n).transpose(1, 0, 2)).astype(dt)


def kernel(input_ids, lambda_val, is_warmup, embed_W, Wqkv_a, Wo_a, Wf1_a, Wf2_a,
           pool_vectors, pool_keys, Wq_ret, W_base, Wqkv_b, Wo_b, Wf1_b, Wf2_b, lm_head_W,
           _trace=False):
    input_ids = np.asarray(input_ids)
    lam = float(np.asarray(lambda_val))
    warm = bool(np.asarray(is_warmup))
    assert input_ids.shape == (4, T)

    key = (lam, warm)
    if key not in _cache:
        _cache[key] = build(lam, warm)
    nc = _cache[key]

    x = np.asarray(embed_W, np.float32)[input_ids] + _sin_pos(T, D)[None]  # [4, T, D]

    keys = np.asarray(pool_keys, np.float32)
    keys_hat = keys / (np.linalg.norm(keys, axis=-1, keepdims=True) + 1e-6)

    tri = np.where(np.arange(P)[None, :] >= np.arange(P)[:, None], 0.0, NEG).astype(np.float32)
    m_own = np.zeros((P, TOWN), np.float32)
    m_own[:, :P] = tri

    shared = {
        "wqkv_a": _fm(Wqkv_a), "wo_a": _fm(Wo_a), "wf1_a": _fm(Wf1_a), "wf2_a": _fm(Wf2_a),
        "wqkv_b": _fm(Wqkv_b), "wo_b": _fm(Wo_b), "wf1_b": _fm(Wf1_b), "wf2_b": _fm(Wf2_b),
        "wq_ret": _fm(Wq_ret, np.float32),
        "keys_t": np.ascontiguousarray(keys_hat.T),
        "w_base_r": _fm(W_base, np.float32),
        "pool_r": np.asarray(pool_vectors, np.float32).astype(ml_dtypes.bfloat16).reshape(NPOOL * D, D),
        "iota_pj": (np.arange(P, dtype=np.float32)[:, None]
                    + 128.0 * np.arange(DT, dtype=np.float32)[None, :]).astype(np.float32),
        "id128": np.eye(P, dtype=np.float32).astype(ml_dtypes.bfloat16),
        "mask_own": m_own.astype(ml_dtypes.bfloat16),
    }

    in_maps = []
    for c in range(NCORES):
        e, s = c // 2, c % 2
        perm = np.concatenate(
            [np.arange((2 * i + s) * P, (2 * i + s + 1) * P) for i in range(4)]
            + [np.arange((2 * i + 1 - s) * P, (2 * i + 2 - s) * P) for i in range(4)])
        xl = np.ascontiguousarray(x[e].T[:, perm])  # [D, T] local order
        m_oth = np.zeros((P, TOWN), np.float32)
        m_oth[:, :P] = 0.0 if s == 1 else NEG
        sel_arr = np.zeros((P, 2), np.float32)
        sel_arr[:, 0] = float(s)
        sel_arr[:, 1] = float(1 - s)
        m = dict(shared)
        m.update({
            "x_fm": _fm(xl),
            "x_own": _fm(xl[:, :TOWN], np.float32),
            "lm_w": _fm(np.asarray(lm_head_W, np.float32)[:, c * VS:(c + 1) * VS]),
            "mask_oth": m_oth.astype(ml_dtypes.bfloat16),
            "selv": sel_arr,
        })
        in_maps.append(m)

    res = run_bass_kernel_spmd(nc, in_maps, core_ids=list(range(NCORES)), trace=_trace)

    logits = np.zeros((4, T, VOC), np.float32)
    for c in range(NCORES):
        oc = res.results[c]["logits_out"].reshape(NCORES, 4, P, VS)
        for r in range(NCORES):
            er, sr = r // 2, r % 2
            for i in range(4):
                logits[er, (2 * i + sr) * P:(2 * i + sr + 1) * P, c * VS:(c + 1) * VS] = oc[r, i]
    alphas = np.stack([res.results[2 * e]["alphas_out"][0] for e in range(4)]).astype(np.float32)
    indices = np.stack([res.results[2 * e]["idx_out"][0] for e in range(4)]).astype(np.int32)
    kernel._last = res
    return logits, alphas, indices
